# revision 14
# baseline (speedup 1.0000x reference)
"""Trainium2 Bass kernel for the LogicLayer problem, v5 (trail chaining with
SBUF->SBUF patch DMAs).

out[n, y] = k0[y] + k1[y]*a + k2[y]*b + k3[y]*(a*b)
  a = x[n, a_idx[y]], b = x[n, b_idx[y]], k = softmax(weights) @ GATE_COEFFS

Outputs are edges (a_idx[y], b_idx[y]) of a multigraph over the 16384
feature rows.  Edges are trail-decomposed and laid along the 16-chunk axis
of each core's 128x16 output grid, so the A-operand of chunk t is chunk
t-1's B-gather at the same partition.  Per chunk we gather 128 new B rows
plus M_t trail-start "fix" rows (dense prefix of the fix slot); small
SBUF->SBUF patch DMAs then copy each fix row onto its (dead) break
partition in the previous B tile, so compute reads one contiguous A tile:

  V = k3*A + k2            (DVE tensor_scalar, 2x packed mode)
  U = k1*A + k0            (ACT on the scalar engine)
  V = V * B                (DVE tensor_tensor)
  O = V + U                (DVE tensor_tensor)

The break pattern (which (partition, chunk) slots start a new trail piece)
is SHARED across all 8 cores so the patch DMA access patterns are
compile-time; each core fills the shared slot-length pattern by cutting its
own trails (always feasible; the pattern is synthesized so that every core
can fill it greedily).  ~2700 gathered rows/core (22 MiB) vs 4096 (32 MiB)
for the a/b-gather baseline.  Host does softmax, the a<->b gate swap for
reversed trail edges (the gate set is closed under operand swap), and the
final row un-permutation + transpose.
"""

import numpy as np

_GATE_COEFFS = np.array(
    [
        [0.0, 0.0, 0.0, 0.0],
        [0.0, 0.0, 0.0, 1.0],
        [0.0, 1.0, 0.0, -1.0],
        [0.0, 1.0, 0.0, 0.0],
        [0.0, 0.0, 1.0, -1.0],
        [0.0, 0.0, 1.0, 0.0],
        [0.0, 1.0, 1.0, -2.0],
        [0.0, 1.0, 1.0, -1.0],
        [1.0, -1.0, -1.0, 1.0],
        [1.0, -1.0, -1.0, 2.0],
        [1.0, 0.0, -1.0, 0.0],
        [1.0, 0.0, -1.0, 1.0],
        [1.0, -1.0, 0.0, 0.0],
        [1.0, -1.0, 0.0, 1.0],
        [1.0, 0.0, 0.0, -1.0],
        [1.0, 0.0, 0.0, 0.0],
    ],
    dtype=np.float32,
)
_SIGMA = np.array([0, 1, 4, 5, 2, 3, 6, 7, 8, 9, 12, 13, 10, 11, 14, 15])

BATCH, IN_DIM, OUT_DIM = 4096, 16384, 16384
NCORES = 8
OC = OUT_DIM // NCORES
NCHUNK = OC // 128

_PROGRAM_CACHE: dict = {}
_PLAN_CACHE: dict = {}


def _decompose_trails(a_idx, b_idx):
    n_edges = len(a_idx)
    adj: dict = {}
    for i in range(n_edges):
        u = int(a_idx[i]); v = int(b_idx[i])
        adj.setdefault(u, []).append((i, v))
        adj.setdefault(v, []).append((i, u))
    used = np.zeros(n_edges, dtype=bool)
    ptr = {u: 0 for u in adj}
    deg = {u: len(lst) for u, lst in adj.items()}

    def walk(start):
        trail = []
        cur = start
        while True:
            lst = adj[cur]
            p = ptr[cur]
            while p < len(lst) and used[lst[p][0]]:
                p += 1
            ptr[cur] = p
            if p >= len(lst):
                break
            eid, nxt = lst[p]
            used[eid] = True
            trail.append((eid, cur, nxt))
            cur = nxt
        return trail

    trails = []
    order = [u for u in adj if deg[u] % 2 == 1] + [
        u for u in adj if deg[u] % 2 == 0
    ]
    for u in order:
        while True:
            t = walk(u)
            if not t:
                break
            trails.append(t)
    assert sum(len(t) for t in trails) == n_edges
    return trails


def _fill_rows_lengths(lengths):
    """Exact bin-fill of slot lengths into 128 rows summing 16 each (cutting
    allowed).  Returns per-row length lists."""
    buckets = [0] * 17
    for L in lengths:
        buckets[L] += 1
    rows = []
    for _ in range(128):
        rem = 16
        row = []
        while rem > 0:
            l = 0
            for L in range(rem, 0, -1):
                if buckets[L]:
                    l = L
                    break
            if l == 0:
                Lmax = max(L for L in range(17) if buckets[L])
                buckets[Lmax] -= 1
                row.append(rem)
                buckets[Lmax - rem] += 1
                rem = 0
            else:
                buckets[l] -= 1
                row.append(l)
                rem -= l
        rows.append(row)
    assert not any(buckets)
    return rows


def _order_rows_lengths(rows, ncols=NCHUNK):
    """Order slot lengths within each row to balance break counts per
    column."""
    col_load = [0] * ncols
    out = []
    for lens in rows:
        remaining = list(lens)
        order = []
        pos = 0
        while remaining:
            best, bestload = None, None
            for i, L in enumerate(remaining):
                nxt = pos + L
                load = col_load[nxt] if nxt < ncols else -1
                if bestload is None or load < bestload:
                    bestload, best = load, i
            L = remaining.pop(best)
            order.append(L)
            pos += L
            if pos < ncols:
                col_load[pos] += 1
        out.append(order)
    return out


def _core_feasible(slot_lengths, trail_lengths):
    """Can slots (desc) be filled by cutting trails (longest-first greedy)?"""
    buckets = [0] * 64
    for L in trail_lengths:
        buckets[min(L, 63)] += 1
    top = 63
    for s in sorted(slot_lengths, reverse=True):
        while top > 0 and not buckets[top]:
            top -= 1
        if top < s:
            return False
        buckets[top] -= 1
        rest = top - s
        if rest:
            buckets[rest] += 1
            if rest > top:
                top = rest
    return True


def _build_plan(a_idx, b_idx):
    a = np.asarray(a_idx).astype(np.int64)
    b = np.asarray(b_idx).astype(np.int64)
    trails = _decompose_trails(a, b)

    # split trails across cores, balancing total length to exactly OC and
    # roughly balancing counts: round-robin longest-first
    trails.sort(key=len, reverse=True)
    core_fill = [0] * NCORES
    core_trails = [[] for _ in range(NCORES)]
    for t in trails:
        i = 0
        while i < len(t):
            c = min(range(NCORES), key=lambda cc: core_fill[cc])
            room = OC - core_fill[c]
            assert room > 0
            seg = t[i : i + room]
            core_trails[c].append(seg)
            core_fill[c] += len(seg)
            i += len(seg)
    assert all(f == OC for f in core_fill)

    # ---- shared slot-length pattern, feasible for every core ----
    core_lens = [[len(t) for t in ct] for ct in core_trails]
    # initial synth: per-length average count (>=2), deficit filled with 1s
    cnt = np.zeros((NCORES, 17), dtype=np.int64)
    for c in range(NCORES):
        for L in core_lens[c]:
            cnt[c][min(L, 16)] += 1
    synth = []
    for L in range(16, 1, -1):
        synth += [L] * int(cnt[:, L].mean())
    mass = sum(synth)
    assert mass <= OC
    synth += [1] * (OC - mass)
    # feasibility loop: while some core cannot fill, split the largest slot
    for _ in range(2048):
        bad = [c for c in range(NCORES) if not _core_feasible(synth, core_lens[c])]
        if not bad:
            break
        synth.sort(reverse=True)
        L = synth.pop(0)
        assert L >= 2, "pattern infeasible even with unit slots"
        synth += [L - 1, 1]
    else:
        raise RuntimeError("pattern synthesis failed")

    # canonical within-row order (desc) and rows sorted by break bitmask:
    # clusters identical/similar break sets so per-chunk break partitions
    # form few contiguous runs (few patch DMA instructions)
    rows_lens = [sorted(r, reverse=True) for r in _fill_rows_lengths(synth)]

    def _brk_key(lens):
        brk = [0] * NCHUNK
        pos = 0
        for L in lens:
            brk[pos] = 1
            pos += L
        return tuple(brk)

    rows_lens.sort(key=_brk_key)

    # shared break pattern
    brkS = np.zeros((128, NCHUNK), dtype=bool)
    for p in range(128):
        pos = 0
        for L in rows_lens[p]:
            brkS[p, pos] = True
            pos += L
        assert pos == NCHUNK
    M = [int(brkS[:, t].sum()) for t in range(NCHUNK)]
    assert M[0] == 128
    reg = [128 + M[t] if t else 256 for t in range(NCHUNK)]
    reg[0] = 256
    ni = [-(-r // 16) * 16 for r in reg]
    cols = [n // 16 for n in ni]

    # patch runs per chunk: maximal consecutive break-partition runs
    runs = []
    for t in range(NCHUNK):
        bp = np.where(brkS[:, t])[0]
        rt = []
        if t >= 1 and len(bp):
            q = 0
            s = int(bp[0]); prev = s
            for p in bp[1:]:
                p = int(p)
                if p == prev + 1:
                    prev = p
                    continue
                rt.append((s, prev - s + 1, q))
                q += prev - s + 1
                s = prev = p
            rt.append((s, prev - s + 1, q))
        runs.append(rt)

    # ---- per-core fill of the shared pattern ----
    # slots sorted by length desc; each filled by cutting the longest trail
    slot_list = []   # (length, p, t)
    for p in range(128):
        pos = 0
        for L in rows_lens[p]:
            slot_list.append((L, p, pos))
            pos += L
    slot_list.sort(key=lambda s: -s[0])

    ia_cores = []
    y_of_row = np.empty((NCORES, OC), dtype=np.int64)
    swap = np.zeros((NCORES, 128, NCHUNK), dtype=bool)
    for c in range(NCORES):
        buckets: dict = {}
        for t in core_trails[c]:
            buckets.setdefault(len(t), []).append(t)
        top = max(buckets)
        grid = [[None] * NCHUNK for _ in range(128)]
        for L, p, pos in slot_list:
            while top > 0 and not buckets.get(top):
                top -= 1
            assert top >= L, "core cannot fill shared pattern"
            tr = buckets[top].pop()
            piece, rest = tr[:L], tr[L:]
            if rest:
                buckets.setdefault(len(rest), []).append(rest)
                if len(rest) > top:
                    top = len(rest)
            for j, e in enumerate(piece):
                grid[p][pos + j] = e
        seqs = []
        for t in range(NCHUNK):
            seq = np.full(ni[t], -1, dtype=np.int16)
            fixq = 0
            for p in range(128):
                y, frm, to = grid[p][t]
                assert (frm == a[y] and to == b[y]) or (
                    frm == b[y] and to == a[y]
                ), (c, p, t)
                seq[p] = to
                y_of_row[c, t * 128 + p] = y
                swap[c, p, t] = frm == b[y] and to == a[y] and a[y] != b[y]
                if brkS[p, t]:
                    seq[128 + fixq] = frm
                    fixq += 1
                else:
                    assert t >= 1 and frm == grid[p][t - 1][2], (c, p, t)
            assert fixq == M[t]
            seqs.append(seq)
        flatseq = np.concatenate(seqs)
        w = np.ascontiguousarray(flatseq.reshape(-1, 16).T)
        ia_cores.append(np.ascontiguousarray(np.tile(w, (8, 1))))
    return {
        "m": tuple(M),
        "reg": reg,
        "ni": ni,
        "cols": cols,
        "runs": runs,
        "ia_cores": ia_cores,
        "y_of_row": y_of_row,
        "swap": swap,
    }


def _get_plan(a_idx, b_idx):
    key = (np.asarray(a_idx).tobytes(), np.asarray(b_idx).tobytes())
    h = hash(key)
    if _PLAN_CACHE.get("key") != h:
        _PLAN_CACHE["key"] = h
        _PLAN_CACHE["plan"] = _build_plan(a_idx, b_idx)
    return _PLAN_CACHE["plan"]


def _build_program(m, ni, reg, cols, runs):
    import concourse.bass as bass  # noqa: F401
    import concourse.tile as tile
    from concourse import bacc, mybir

    f32 = mybir.dt.float32
    f16 = mybir.dt.float16
    i16 = mybir.dt.int16
    AF = mybir.ActivationFunctionType
    ALU = mybir.AluOpType

    total_cols = sum(cols)
    nc = bacc.Bacc("TRN2", target_bir_lowering=False, debug=False)
    xT_h = nc.dram_tensor("xT", [IN_DIM, BATCH], f16, kind="ExternalInput")
    ia_h = nc.dram_tensor("ia", [128, total_cols], i16, kind="ExternalInput")
    kg_h = nc.dram_tensor("kg", [128, 4 * NCHUNK], f32, kind="ExternalInput")
    out_h = nc.dram_tensor("outT", [OC, BATCH], f16, kind="ExternalOutput")

    with tile.TileContext(nc) as tc:
        from contextlib import ExitStack

        with ExitStack() as stack:
            cp = stack.enter_context(tc.tile_pool(name="const", bufs=1))

            ia_sb = cp.tile([128, total_cols], i16)
            nc.sync.dma_start(ia_sb[:], ia_h.ap()[:, :])
            kg_sb = cp.tile([128, 4 * NCHUNK], f32, tag="kg")
            nc.sync.dma_start(kg_sb[:], kg_h.ap()[:, :])
            zi = cp.tile([128, 1], i16, tag="zi")
            nc.gpsimd.memset(zi[:], 0)
            warm = cp.tile([128, 1, BATCH], f16, tag="warm")

            outT_ap = out_h.ap().rearrange("(c p) n -> p c n", p=128)
            with (
                tc.tile_pool(name="pg", bufs=4) as pg,
                tc.tile_pool(name="po", bufs=3) as po,
                tc.tile_pool(name="pv", bufs=5) as pv,
                tc.tile_pool(name="pu", bufs=3) as pu,
            ):
                nc.gpsimd.dma_gather(
                    out_ap=warm[:, 0:1, :],
                    in_ap=xT_h.ap()[:, :],
                    idxs_ap=zi[:],
                    num_idxs=16,
                    num_idxs_reg=16,
                    elem_size=BATCH,
                    single_packet=False,
                )
                G_prev = None
                c0 = 0
                for t in range(NCHUNK):
                    G = pg.tile([128, 2, BATCH], f16, tag="G")
                    gout = G[:, 0:1, :] if ni[t] <= 128 else G[:]
                    nc.gpsimd.dma_gather(
                        out_ap=gout,
                        in_ap=xT_h.ap()[:, :],
                        idxs_ap=ia_sb[:, c0 : c0 + cols[t]],
                        num_idxs=ni[t],
                        num_idxs_reg=reg[t],
                        elem_size=BATCH,
                        single_packet=False,
                    )
                    c0 += cols[t]

                    if t >= 1:
                        for (p0, L, q0) in runs[t]:
                            nc.sync.dma_start(
                                G_prev[p0 : p0 + L, 0, :],
                                G[q0 : q0 + L, 1, :],
                            )
                        A = G_prev[:, 0, :]
                    else:
                        A = G[:, 1, :]
                    B = G[:, 0, :]
                    # out = (k3*B + k1)*A + (k2*B + k0): B is consumed by the
                    # two early affine ops right after its gather, so the
                    # patch DMAs for the NEXT chunk (which overwrite dead B
                    # rows) only wait on them and their latency hides under
                    # this chunk's tensor_tensor ops.
                    S = pu.tile([128, BATCH], f16, tag="S")
                    W = pv.tile([128, BATCH], f16, tag="W")
                    O = po.tile([128, 1, BATCH], f16, tag="O")
                    nc.scalar.activation(
                        S[:],
                        B,
                        AF.Identity,
                        bias=kg_sb[:, 1 * NCHUNK + t : 1 * NCHUNK + t + 1],
                        scale=kg_sb[:, 3 * NCHUNK + t : 3 * NCHUNK + t + 1],
                    )
                    nc.vector.tensor_scalar(
                        W[:],
                        B,
                        kg_sb[:, 2 * NCHUNK + t : 2 * NCHUNK + t + 1],
                        kg_sb[:, 0 * NCHUNK + t : 0 * NCHUNK + t + 1],
                        ALU.mult,
                        ALU.add,
                    )
                    M = pv.tile([128, BATCH], f16, tag="M")
                    nc.vector.tensor_mul(M[:], S[:], A)
                    nc.vector.tensor_add(O[:, 0, :], M[:], W[:])
                    nc.sync.dma_start(outT_ap[:, t : t + 1, :], O[:, :, :])
                    G_prev = G

    nc.compile()
    return nc


def _host_inputs(x, weights, a_idx, b_idx):
    plan = _get_plan(a_idx, b_idx)
    weights = np.asarray(weights, dtype=np.float32)
    xT16 = np.ascontiguousarray(
        np.asarray(x, dtype=np.float32).T.astype(np.float16)
    )
    w = weights - weights.max(axis=1, keepdims=True)
    e = np.exp(w)
    P = e / e.sum(axis=1, keepdims=True)
    K0 = P @ _GATE_COEFFS
    K1 = P @ _GATE_COEFFS[_SIGMA]
    y_of_row = plan["y_of_row"]
    swap = plan["swap"]
    in_maps = []
    for c in range(NCORES):
        yr = y_of_row[c].reshape(NCHUNK, 128)
        kc = np.where(swap[c].transpose(1, 0)[:, :, None], K1[yr], K0[yr])
        kg = np.ascontiguousarray(
            kc.transpose(1, 2, 0).reshape(128, 4 * NCHUNK)
        ).astype(np.float32)
        in_maps.append({"xT": xT16, "ia": plan["ia_cores"][c], "kg": kg})
    return in_maps


def kernel(x, weights, a_idx, b_idx):
    from concourse.bass_utils import run_bass_kernel_spmd

    plan = _get_plan(a_idx, b_idx)
    pkey = (plan["m"], tuple(tuple(r) for r in map(tuple, plan["runs"])))
    if _PROGRAM_CACHE.get("mkey") != pkey:
        _PROGRAM_CACHE["mkey"] = pkey
        _PROGRAM_CACHE["nc"] = _build_program(
            plan["m"], plan["ni"], plan["reg"], plan["cols"], plan["runs"]
        )
    nc = _PROGRAM_CACHE["nc"]

    in_maps = _host_inputs(x, weights, a_idx, b_idx)
    res = run_bass_kernel_spmd(nc, in_maps, list(range(NCORES)))
    outT = np.concatenate(
        [res.results[c]["outT"] for c in range(NCORES)], axis=0
    )
    y_all = plan["y_of_row"].reshape(-1)
    full = np.empty_like(outT)
    full[y_all] = outT
    return np.ascontiguousarray(full.T).astype(np.float32)


# revision 15
# speedup vs baseline: 1.2009x; 1.2009x over previous
"""Trainium2 Bass kernel for the LogicLayer problem, v6.

out[n, y] = k0[y] + k1[y]*a + k2[y]*b + k3[y]*(a*b)
  a = x[n, a_idx[y]], b = x[n, b_idx[y]], k = softmax(weights) @ GATE_COEFFS

Trail chaining (v5) + multi-queue SWDGE + paired gather calls:

  * Outputs are edges of a multigraph over feature rows; trails are laid
    along the 16-chunk axis so chunk t's A-operand reuses chunk t-1's
    B-gather rows in place.  Only 128 B rows + M_t trail-start fix rows are
    gathered per chunk (~2770 rows/core = 22.7 MiB vs 32 MiB baseline).
  * Gather calls cover TWO chunks each (8 calls instead of 16) and are
    spread round-robin over 4 SWDGE queues so descriptor generation and
    queue drain overlap across calls.
  * Fix rows land densely in the call's fix slots; small compile-time
    SBUF->SBUF patch DMAs (issued on the scalar engine's HWDGE queue) copy
    them onto their (dead) break partitions in the B tile they augment.
    The break pattern is shared by all cores (SPMD single program) and rows
    are sorted by break-set so per-chunk break partitions form few runs.
  * Compute consumes B early so the next patches only wait on the cheap
    affine ops:  S = k3*B + k1 (ACT)   W = k2*B + k0 (DVE tensor_scalar 2x)
                 M = S*A (DVE)         O = M + W     (DVE)
  * Host: softmax + gate collapse, a<->b gate swap for reversed trail edges
    (gate set closed under operand swap), output row un-permute + transpose.
"""

import numpy as np

_GATE_COEFFS = np.array(
    [
        [0.0, 0.0, 0.0, 0.0],
        [0.0, 0.0, 0.0, 1.0],
        [0.0, 1.0, 0.0, -1.0],
        [0.0, 1.0, 0.0, 0.0],
        [0.0, 0.0, 1.0, -1.0],
        [0.0, 0.0, 1.0, 0.0],
        [0.0, 1.0, 1.0, -2.0],
        [0.0, 1.0, 1.0, -1.0],
        [1.0, -1.0, -1.0, 1.0],
        [1.0, -1.0, -1.0, 2.0],
        [1.0, 0.0, -1.0, 0.0],
        [1.0, 0.0, -1.0, 1.0],
        [1.0, -1.0, 0.0, 0.0],
        [1.0, -1.0, 0.0, 1.0],
        [1.0, 0.0, 0.0, -1.0],
        [1.0, 0.0, 0.0, 0.0],
    ],
    dtype=np.float32,
)
_SIGMA = np.array([0, 1, 4, 5, 2, 3, 6, 7, 8, 9, 12, 13, 10, 11, 14, 15])

BATCH, IN_DIM, OUT_DIM = 4096, 16384, 16384
NCORES = 8
OC = OUT_DIM // NCORES
NCHUNK = OC // 128
NCALL = NCHUNK // 2
NQ = 4

_PROGRAM_CACHE: dict = {}
_PLAN_CACHE: dict = {}


def _decompose_trails(a_idx, b_idx):
    n_edges = len(a_idx)
    adj: dict = {}
    for i in range(n_edges):
        u = int(a_idx[i]); v = int(b_idx[i])
        adj.setdefault(u, []).append((i, v))
        adj.setdefault(v, []).append((i, u))
    used = np.zeros(n_edges, dtype=bool)
    ptr = {u: 0 for u in adj}
    deg = {u: len(lst) for u, lst in adj.items()}

    def walk(start):
        trail = []
        cur = start
        while True:
            lst = adj[cur]
            p = ptr[cur]
            while p < len(lst) and used[lst[p][0]]:
                p += 1
            ptr[cur] = p
            if p >= len(lst):
                break
            eid, nxt = lst[p]
            used[eid] = True
            trail.append((eid, cur, nxt))
            cur = nxt
        return trail

    trails = []
    order = [u for u in adj if deg[u] % 2 == 1] + [
        u for u in adj if deg[u] % 2 == 0
    ]
    for u in order:
        while True:
            t = walk(u)
            if not t:
                break
            trails.append(t)
    assert sum(len(t) for t in trails) == n_edges
    return trails


def _fill_rows_lengths(lengths):
    buckets = [0] * 17
    for L in lengths:
        buckets[L] += 1
    rows = []
    for _ in range(128):
        rem = 16
        row = []
        while rem > 0:
            l = 0
            for L in range(rem, 0, -1):
                if buckets[L]:
                    l = L
                    break
            if l == 0:
                Lmax = max(L for L in range(17) if buckets[L])
                buckets[Lmax] -= 1
                row.append(rem)
                buckets[Lmax - rem] += 1
                rem = 0
            else:
                buckets[l] -= 1
                row.append(l)
                rem -= l
        rows.append(row)
    assert not any(buckets)
    return rows


def _core_feasible(slot_lengths, trail_lengths):
    buckets = [0] * 64
    for L in trail_lengths:
        buckets[min(L, 63)] += 1
    top = 63
    for s in sorted(slot_lengths, reverse=True):
        while top > 0 and not buckets[top]:
            top -= 1
        if top < s:
            return False
        buckets[top] -= 1
        rest = top - s
        if rest:
            buckets[rest] += 1
    return True


def _build_plan(a_idx, b_idx):
    a = np.asarray(a_idx).astype(np.int64)
    b = np.asarray(b_idx).astype(np.int64)
    trails = _decompose_trails(a, b)

    trails.sort(key=len, reverse=True)
    core_fill = [0] * NCORES
    core_trails = [[] for _ in range(NCORES)]
    for t in trails:
        i = 0
        while i < len(t):
            c = min(range(NCORES), key=lambda cc: core_fill[cc])
            room = OC - core_fill[c]
            assert room > 0
            seg = t[i : i + room]
            core_trails[c].append(seg)
            core_fill[c] += len(seg)
            i += len(seg)
    assert all(f == OC for f in core_fill)

    # shared slot-length pattern, feasible for every core
    core_lens = [[len(t) for t in ct] for ct in core_trails]
    cnt = np.zeros((NCORES, 17), dtype=np.int64)
    for c in range(NCORES):
        for L in core_lens[c]:
            cnt[c][min(L, 16)] += 1
    synth = []
    for L in range(16, 1, -1):
        synth += [L] * int(cnt[:, L].mean())
    mass = sum(synth)
    assert mass <= OC
    synth += [1] * (OC - mass)
    for _ in range(2048):
        bad = [c for c in range(NCORES) if not _core_feasible(synth, core_lens[c])]
        if not bad:
            break
        synth.sort(reverse=True)
        L = synth.pop(0)
        assert L >= 2
        synth += [L - 1, 1]
    else:
        raise RuntimeError("pattern synthesis failed")

    # canonical (desc) within-row, rows sorted by break bitmask -> few runs
    rows_lens = [sorted(r, reverse=True) for r in _fill_rows_lengths(synth)]

    def _brk_key(lens):
        brk = [0] * NCHUNK
        pos = 0
        for L in lens:
            brk[pos] = 1
            pos += L
        return tuple(brk)

    rows_lens.sort(key=_brk_key)

    brkS = np.zeros((128, NCHUNK), dtype=bool)
    for p in range(128):
        pos = 0
        for L in rows_lens[p]:
            brkS[p, pos] = True
            pos += L
        assert pos == NCHUNK
    M = [int(brkS[:, t].sum()) for t in range(NCHUNK)]
    assert M[0] == 128

    # per-call geometry: call c covers chunks 2c, 2c+1
    reg, ni, cols = [], [], []
    for c in range(NCALL):
        r = 256 + M[2 * c] + M[2 * c + 1]
        reg.append(r)
        ni.append(-(-r // 16) * 16)
        cols.append(ni[-1] // 16)

    # patch table: per chunk, list of (dst_p0, L, src_slot, src_part)
    # fix q of chunk t lives at call-flat position base+q ->
    #   slot 2 + (base+q)//128, partition (base+q)%128
    patches = []
    for t in range(NCHUNK):
        base = 0 if t % 2 == 0 else M[t - 1]
        bp = np.where(brkS[:, t])[0]
        plist = []
        if t >= 1 and len(bp):
            # maximal dst runs
            runsl = []
            s = int(bp[0]); prev = s; q = 0
            for p in bp[1:]:
                p = int(p)
                if p == prev + 1:
                    prev = p
                    continue
                runsl.append((s, prev - s + 1, q))
                q += prev - s + 1
                s = prev = p
            runsl.append((s, prev - s + 1, q))
            # split each at fix-slot partition wrap
            for (p0, L, q0) in runsl:
                off = 0
                while L > 0:
                    fl = base + q0 + off
                    slot = 2 + fl // 128
                    part = fl % 128
                    take = min(L, 128 - part)
                    plist.append((p0 + off, take, slot, part))
                    off += take
                    L -= take
        patches.append(plist)

    # per-core fill of the shared pattern
    slot_list = []
    for p in range(128):
        pos = 0
        for L in rows_lens[p]:
            slot_list.append((L, p, pos))
            pos += L
    slot_list.sort(key=lambda s: -s[0])

    ia_cores = []
    y_of_row = np.empty((NCORES, OC), dtype=np.int64)
    swap = np.zeros((NCORES, 128, NCHUNK), dtype=bool)
    for c in range(NCORES):
        buckets: dict = {}
        for t in core_trails[c]:
            buckets.setdefault(len(t), []).append(t)
        top = max(buckets)
        grid = [[None] * NCHUNK for _ in range(128)]
        for L, p, pos in slot_list:
            while top > 0 and not buckets.get(top):
                top -= 1
            assert top >= L, "core cannot fill shared pattern"
            tr = buckets[top].pop()
            piece, rest = tr[:L], tr[L:]
            if rest:
                buckets.setdefault(len(rest), []).append(rest)
            for j, e in enumerate(piece):
                grid[p][pos + j] = e
        seqs = []
        for cl in range(NCALL):
            seq = np.full(ni[cl], -1, dtype=np.int16)
            fixbase = 256
            for half in range(2):
                t = 2 * cl + half
                fixq = 0
                for p in range(128):
                    y, frm, to = grid[p][t]
                    assert (frm == a[y] and to == b[y]) or (
                        frm == b[y] and to == a[y]
                    ), (c, p, t)
                    seq[half * 128 + p] = to
                    y_of_row[c, t * 128 + p] = y
                    swap[c, p, t] = frm == b[y] and to == a[y] and a[y] != b[y]
                    if brkS[p, t]:
                        seq[fixbase + fixq] = frm
                        fixq += 1
                    else:
                        assert t >= 1 and frm == grid[p][t - 1][2], (c, p, t)
                assert fixq == M[t]
                fixbase += fixq
            assert fixbase == reg[cl]
            seqs.append(seq)
        flatseq = np.concatenate(seqs)
        w = np.ascontiguousarray(flatseq.reshape(-1, 16).T)
        ia_cores.append(np.ascontiguousarray(np.tile(w, (8, 1))))
    return {
        "m": tuple(M),
        "reg": reg,
        "ni": ni,
        "cols": cols,
        "patches": patches,
        "ia_cores": ia_cores,
        "y_of_row": y_of_row,
        "swap": swap,
    }


def _get_plan(a_idx, b_idx):
    key = (np.asarray(a_idx).tobytes(), np.asarray(b_idx).tobytes())
    h = hash(key)
    if _PLAN_CACHE.get("key") != h:
        _PLAN_CACHE["key"] = h
        _PLAN_CACHE["plan"] = _build_plan(a_idx, b_idx)
    return _PLAN_CACHE["plan"]


def _build_program(m, ni, reg, cols, patches):
    import concourse.bass as bass  # noqa: F401
    import concourse.tile as tile
    from concourse import bacc, mybir

    f32 = mybir.dt.float32
    f16 = mybir.dt.float16
    i16 = mybir.dt.int16
    AF = mybir.ActivationFunctionType
    ALU = mybir.AluOpType

    total_cols = sum(cols)
    nc = bacc.Bacc(
        "TRN2", target_bir_lowering=False, debug=False, num_swdge_queues=NQ
    )
    xT_h = nc.dram_tensor("xT", [IN_DIM, BATCH], f16, kind="ExternalInput")
    ia_h = nc.dram_tensor("ia", [128, total_cols], i16, kind="ExternalInput")
    kg_h = nc.dram_tensor("kg", [128, 4 * NCHUNK], f32, kind="ExternalInput")
    out_h = nc.dram_tensor("outT", [OC, BATCH], f16, kind="ExternalOutput")

    with tile.TileContext(nc) as tc:
        from contextlib import ExitStack

        with ExitStack() as stack:
            cp = stack.enter_context(tc.tile_pool(name="const", bufs=1))

            ia_sb = cp.tile([128, total_cols], i16)
            nc.sync.dma_start(ia_sb[:], ia_h.ap()[:, :])
            kg_sb = cp.tile([128, 4 * NCHUNK], f32, tag="kg")
            nc.sync.dma_start(kg_sb[:], kg_h.ap()[:, :])
            zi = cp.tile([128, 1], i16, tag="zi")
            nc.gpsimd.memset(zi[:], 0)

            outT_ap = out_h.ap().rearrange("(c p) n -> p c n", p=128)
            with (
                tc.tile_pool(name="pg", bufs=3) as pg,
                tc.tile_pool(name="po", bufs=3) as po,
                tc.tile_pool(name="pv", bufs=4) as pv,
                tc.tile_pool(name="pu", bufs=2) as pu,
            ):
                # warm up all SWDGE queues (library load overlaps const DMA)
                Gw = pg.tile([128, 4, BATCH], f16, tag="G", name="Gwarm")
                for q in range(NQ):
                    nc.gpsimd.dma_gather(
                        out_ap=Gw[:, q : q + 1, :],
                        in_ap=xT_h.ap()[:, :],
                        idxs_ap=zi[:],
                        num_idxs=16,
                        num_idxs_reg=16,
                        elem_size=BATCH,
                        single_packet=False,
                        queue_num=q,
                    )
                G_prev = None
                c0 = 0
                for cl in range(NCALL):
                    G = pg.tile([128, 4, BATCH], f16, tag="G")
                    nslots = -(-ni[cl] // 128)
                    nc.gpsimd.dma_gather(
                        out_ap=G[:, 0:nslots, :] if nslots < 4 else G[:],
                        in_ap=xT_h.ap()[:, :],
                        idxs_ap=ia_sb[:, c0 : c0 + cols[cl]],
                        num_idxs=ni[cl],
                        num_idxs_reg=reg[cl],
                        elem_size=BATCH,
                        single_packet=False,
                        queue_num=cl % NQ,
                    )
                    c0 += cols[cl]

                    for half in range(2):
                        t = 2 * cl + half
                        # patch fix rows onto dead break partitions of the
                        # tile that provides this chunk's A operand
                        if t == 0:
                            A = G[:, 2, :]      # identity-ordered fixes
                        elif half == 0:
                            for (p0, L, slot, part) in patches[t]:
                                nc.scalar.dma_start(
                                    G_prev[p0 : p0 + L, 1, :],
                                    G[part : part + L, slot, :],
                                )
                            A = G_prev[:, 1, :]
                        else:
                            for (p0, L, slot, part) in patches[t]:
                                nc.scalar.dma_start(
                                    G[p0 : p0 + L, 0, :],
                                    G[part : part + L, slot, :],
                                )
                            A = G[:, 0, :]
                        B = G[:, half, :]
                        S = pu.tile([128, BATCH], f16, tag="S")
                        W = pv.tile([128, BATCH], f16, tag="W")
                        O = po.tile([128, 1, BATCH], f16, tag="O")
                        nc.scalar.activation(
                            S[:],
                            B,
                            AF.Identity,
                            bias=kg_sb[:, 1 * NCHUNK + t : 1 * NCHUNK + t + 1],
                            scale=kg_sb[:, 3 * NCHUNK + t : 3 * NCHUNK + t + 1],
                        )
                        nc.vector.tensor_scalar(
                            W[:],
                            B,
                            kg_sb[:, 2 * NCHUNK + t : 2 * NCHUNK + t + 1],
                            kg_sb[:, 0 * NCHUNK + t : 0 * NCHUNK + t + 1],
                            ALU.mult,
                            ALU.add,
                        )
                        M_ = pv.tile([128, BATCH], f16, tag="M")
                        nc.vector.tensor_mul(M_[:], S[:], A)
                        nc.vector.tensor_add(O[:, 0, :], M_[:], W[:])
                        nc.sync.dma_start(
                            outT_ap[:, t : t + 1, :], O[:, :, :]
                        )
                    G_prev = G

    nc.compile()
    return nc


def _host_inputs(x, weights, a_idx, b_idx):
    plan = _get_plan(a_idx, b_idx)
    weights = np.asarray(weights, dtype=np.float32)
    xT16 = np.ascontiguousarray(
        np.asarray(x, dtype=np.float32).T.astype(np.float16)
    )
    w = weights - weights.max(axis=1, keepdims=True)
    e = np.exp(w)
    P = e / e.sum(axis=1, keepdims=True)
    K0 = P @ _GATE_COEFFS
    K1 = P @ _GATE_COEFFS[_SIGMA]
    y_of_row = plan["y_of_row"]
    swap = plan["swap"]
    in_maps = []
    for c in range(NCORES):
        yr = y_of_row[c].reshape(NCHUNK, 128)
        kc = np.where(swap[c].transpose(1, 0)[:, :, None], K1[yr], K0[yr])
        kg = np.ascontiguousarray(
            kc.transpose(1, 2, 0).reshape(128, 4 * NCHUNK)
        ).astype(np.float32)
        in_maps.append({"xT": xT16, "ia": plan["ia_cores"][c], "kg": kg})
    return in_maps


def kernel(x, weights, a_idx, b_idx):
    from concourse.bass_utils import run_bass_kernel_spmd

    plan = _get_plan(a_idx, b_idx)
    pkey = (plan["m"], tuple(tuple(p) for p in map(tuple, plan["patches"])))
    if _PROGRAM_CACHE.get("mkey") != pkey:
        _PROGRAM_CACHE["mkey"] = pkey
        _PROGRAM_CACHE["nc"] = _build_program(
            plan["m"], plan["ni"], plan["reg"], plan["cols"], plan["patches"]
        )
    nc = _PROGRAM_CACHE["nc"]

    in_maps = _host_inputs(x, weights, a_idx, b_idx)
    res = run_bass_kernel_spmd(nc, in_maps, list(range(NCORES)))
    outT = np.concatenate(
        [res.results[c]["outT"] for c in range(NCORES)], axis=0
    )
    y_all = plan["y_of_row"].reshape(-1)
    full = np.empty_like(outT)
    full[y_all] = outT
    return np.ascontiguousarray(full.T).astype(np.float32)
